# revision 20
# baseline (speedup 1.0000x reference)
"""Windowed attention (swin-style, 49-token windows, 8 heads) with DynamicPosBias.

Data-parallel over B=2048 windows -> 256 windows/core on 8 cores.

Device does only the attention core; everything cheap runs on host:
- host computes the DynamicPosBias MLP (169x32, microseconds) and ships
  E = exp(rpb) as a [128, 196] f16 table (multiplicative softmax bias),
  zero outside the valid key rows.
- host pre-transposes q, k, v; host normalizes the unnormalized device
  output using the denominator column the device emits per head.

Key-row convention within a head pair (partition axis): even head keys at
rows 0:49, odd head keys at rows 64:113 (compute-engine APs must start at
32-aligned partitions; matmul stationary operands need one contiguous
free dim).

Per window on device:
- 8 QK matmuls (stationary K_h [64, 49], moving Q_h [64, 49]) into
  S^T PSUM [128, 392] (two windows share one PSUM tile; rows 49:64 and
  113:128 stay zero).
- one exp (scale 1/8, ACT) + one Pool multiply by E -> exm [128, 392] f16.
- 4 PV matmuls: stationary exm block [128, 49], moving V-pair [128, 2, 65]
  (65th column of each head's V is ones) -> out [49, 130] token-major with
  the softmax denominator in column 64 of each head block.
- f32->f16 copies of the PV PSUM split across ACT / DVE.
DMA in 16-window groups (6 dma_starts per group) issued from the sync and
scalar sequencers (~700 ns fixed issue cost each).
"""

import numpy as np
from contextlib import ExitStack

import concourse.bass as bass
import concourse.mybir as mybir
import concourse.tile as tile
from concourse import bacc
from concourse.bass_utils import run_bass_kernel_spmd

G = 7
NTOK = 49          # tokens per window
H = 8              # heads
HD = 64            # head dim
C = 512
B = 2048
NCORES = 8
W = B // NCORES    # windows per core = 256
GRP = 16           # windows per DMA group
NG = W // GRP      # 16 groups
NPAIR = GRP // 2   # window pairs per group
NEX = 6            # exm slots
F32 = mybir.dt.float32
F16 = mybir.dt.float16

_CACHED_NC = None
LAST_RESULTS = None


def _rel_idx():
    coords = np.stack(np.meshgrid(np.arange(G), np.arange(G), indexing="ij")).reshape(2, -1)
    rel = (coords[:, :, None] - coords[:, None, :]).transpose(1, 2, 0).copy()
    rel[:, :, 0] += G - 1
    rel[:, :, 1] += G - 1
    rel[:, :, 0] *= 2 * G - 1
    return rel.sum(-1)  # [t, j] in [0, 169)


def _ln(x, g, b, eps=1e-5):
    mu = x.mean(-1, keepdims=True)
    var = ((x - mu) ** 2).mean(-1, keepdims=True)
    return (x - mu) / np.sqrt(var + eps) * g + b


def _host_pos_mlp(pos_proj_w, pos_proj_b, ln1_g, ln1_b, w1, b1,
                  ln2_g, ln2_b, w2, b2, ln3_g, ln3_b, w3, b3):
    pb = np.arange(1 - G, G, dtype=np.float64)
    biases = np.stack(np.meshgrid(pb, pb, indexing="ij")).reshape(2, -1).T  # [169, 2]
    pos = biases @ pos_proj_w + pos_proj_b
    pos = np.maximum(_ln(pos, ln1_g, ln1_b), 0.0) @ w1 + b1
    pos = np.maximum(_ln(pos, ln2_g, ln2_b), 0.0) @ w2 + b2
    pos = np.maximum(_ln(pos, ln3_g, ln3_b), 0.0) @ w3 + b3  # [169, 8]
    rpb = pos[_rel_idx()]            # [49, 49, 8] = (t, j, h)
    rpbr = rpb.transpose(2, 0, 1).reshape(4, 2, NTOK, NTOK)  # (i, p, t, j)
    Eh = np.exp(rpbr).transpose(1, 3, 0, 2)  # (p, j, i, t)
    E = np.zeros((128, 4 * NTOK), np.float32)
    E[0:NTOK] = Eh[0].reshape(NTOK, 4 * NTOK)
    E[64:64 + NTOK] = Eh[1].reshape(NTOK, 4 * NTOK)
    return np.ascontiguousarray(E)


def _build_nc():
    global _CACHED_NC
    if _CACHED_NC is not None:
        return _CACHED_NC
    nc = bacc.Bacc(None, target_bir_lowering=False)

    qp_d = nc.dram_tensor("qp", [NG, 128, GRP * 196], F16, kind="ExternalInput")
    kpe_d = nc.dram_tensor("kpe", [NG, 64, GRP * 196], F16, kind="ExternalInput")
    kpo_d = nc.dram_tensor("kpo", [NG, 64, GRP * 196], F16, kind="ExternalInput")
    vpe_d = nc.dram_tensor("vpe", [NG, NTOK, GRP * 260], F16, kind="ExternalInput")
    vpo_d = nc.dram_tensor("vpo", [NG, NTOK, GRP * 260], F16, kind="ExternalInput")
    e_d = nc.dram_tensor("etab", [128, 196], F16, kind="ExternalInput")
    ot_d = nc.dram_tensor("ot", [NG, NTOK, GRP * 520], F16, kind="ExternalOutput")

    with tile.TileContext(nc) as tc, ExitStack() as ctx:
        const = ctx.enter_context(tc.tile_pool(name="const", bufs=1))
        expool = ctx.enter_context(tc.tile_pool(name="expool", bufs=4))
        stp = ctx.enter_context(tc.tile_pool(name="stp", bufs=1, space="PSUM"))
        pvp = ctx.enter_context(tc.tile_pool(name="pvp", bufs=2, space="PSUM"))

        e_sb = const.tile([128, 196], F16, tag="etab")
        nc.sync.dma_start(e_sb[:], e_d[:])

        # two persistent S^T PSUM tiles; rows 49:64, 113:128 are never
        # written by the QK matmuls and must read as zero for exp
        st_ab = []
        for s in range(3):
            st = stp.tile([128, 392], F32, tag=f"st{s}", name=f"st{s}")
            nc.vector.memset(st[:], 0.0)
            st_ab.append(st)

        qt_s, kt_s, vt_s, ot_s = [], [], [], []
        for s in range(3):
            qt = const.tile([128, GRP * 196], F16, tag=f"qt{s}", name=f"qt{s}")
            kt = const.tile([128, 2 * GRP * 196], F16, tag=f"kt{s}", name=f"kt{s}")
            vt = const.tile([128, 2 * GRP * 260], F16, tag=f"vt{s}", name=f"vt{s}")
            ot = const.tile([NTOK, GRP * 520], F16, tag=f"ot{s}", name=f"ot{s}")
            # V-pair moving operand: off-parity quadrants must be zero
            nc.gpsimd.memset(vt[:], 0.0)
            qt_s.append(qt); kt_s.append(kt); vt_s.append(vt); ot_s.append(ot)

        exm_s = [const.tile([128, 392], F16, tag=f"exm{s}", name=f"exm{s}")
                 for s in range(NEX)]

        ktq_s = [None, None, None]
        vtq_s = [None, None, None]

        def _emit_pv(p):
            gg, wpp = divmod(p, NPAIR)
            ss = gg % 3
            ott = ot_s[gg % 2]
            exm = exm_s[p % NEX]
            for w01 in (0, 1):
                w = 2 * wpp + w01
                pva = pvp.tile([NTOK, 260], F32, tag="pva")
                pvb = pvp.tile([NTOK, 260], F32, tag="pvb")
                for i in range(4):
                    dst = pva if i < 2 else pvb
                    nc.tensor.matmul(
                        out=dst[:, 130 * (i % 2): 130 * (i % 2 + 1)],
                        lhsT=exm[:, w01 * 196 + 49 * i: w01 * 196 + 49 * (i + 1)],
                        rhs=vtq_s[ss][:, w, i, :, :],
                        start=True, stop=True)
                # f32 -> f16 copies, split across ACT / DVE
                nc.scalar.copy(ott[:, w * 520: w * 520 + 260], pva[:])
                nc.vector.tensor_copy(ott[:, w * 520 + 260: (w + 1) * 520], pvb[:])
            if wpp == NPAIR - 1:
                OC = GRP * 520 // 4
                for c4 in range(4):
                    eng = nc.gpsimd if c4 < 2 else (nc.sync if c4 == 2 else nc.scalar)
                    eng.dma_start(ot_d[gg][:, c4 * OC: (c4 + 1) * OC],
                                  ott[:, c4 * OC: (c4 + 1) * OC])

        for g in range(NG):
            s = g % 3
            qt, kt, vt, ot = qt_s[s], kt_s[s], vt_s[s], ot_s[g % 2]
            # input DMAs: split into chunks so descriptors fan out across
            # the 16 DMA queues; all runs are multi-KB contiguous
            QC = GRP * 196 // 4
            for c4 in range(4):
                eng = nc.sync if c4 % 2 == 0 else nc.scalar
                eng.dma_start(qt[:, c4 * QC: (c4 + 1) * QC],
                              qp_d[g][:, c4 * QC: (c4 + 1) * QC])
            KC = GRP * 196 // 2
            for c2 in range(2):
                nc.sync.dma_start(kt[0:64, c2 * KC: (c2 + 1) * KC],
                                  kpe_d[g][:, c2 * KC: (c2 + 1) * KC])
                nc.scalar.dma_start(kt[64:128, GRP * 196 + c2 * KC: GRP * 196 + (c2 + 1) * KC],
                                    kpo_d[g][:, c2 * KC: (c2 + 1) * KC])
            VC = GRP * 260 // 2
            for c2 in range(2):
                nc.sync.dma_start(vt[0:NTOK, c2 * VC: (c2 + 1) * VC],
                                  vpe_d[g][:, c2 * VC: (c2 + 1) * VC])
                nc.scalar.dma_start(vt[64:64 + NTOK, GRP * 260 + c2 * VC: GRP * 260 + (c2 + 1) * VC],
                                    vpo_d[g][:, c2 * VC: (c2 + 1) * VC])

            ktq_s[s] = kt[:].rearrange("p (h w i t) -> p h w i t", h=2, w=GRP, i=4)
            vtq_s[s] = vt[:].rearrange("p (h w i c) -> p w i h c", h=2, w=GRP, i=4)

            for wp in range(NPAIR):
                p = g * NPAIR + wp
                st = st_ab[p % 3]
                for w01 in (0, 1):
                    w = 2 * wp + w01
                    for i in range(4):
                        nc.tensor.matmul(
                            out=st[0:NTOK, w01 * 196 + 49 * i: w01 * 196 + 49 * (i + 1)],
                            lhsT=ktq_s[s][0:64, 0, w, i, :],
                            rhs=qt[0:64, w * 196 + 49 * i: w * 196 + 49 * (i + 1)],
                            start=True, stop=True)
                        nc.tensor.matmul(
                            out=st[64:64 + NTOK, w01 * 196 + 49 * i: w01 * 196 + 49 * (i + 1)],
                            lhsT=ktq_s[s][64:128, 1, w, i, :],
                            rhs=qt[64:128, w * 196 + 49 * i: w * 196 + 49 * (i + 1)],
                            start=True, stop=True)
                ex = expool.tile([128, 392], F16, tag="ex")
                nc.scalar.activation(ex[:], st[:], mybir.ActivationFunctionType.Exp,
                                     scale=0.125)
                exm = exm_s[p % NEX]
                nc.vector.tensor_tensor(
                    out=exm[:].rearrange("p (o c) -> p o c", o=2),
                    in0=ex[:].rearrange("p (o c) -> p o c", o=2),
                    in1=e_sb[:].rearrange("p (o c) -> p o c", o=1).to_broadcast([128, 2, 196]),
                    op=mybir.AluOpType.mult)
                # software pipelining: emit PV for the PREVIOUS pair so the
                # in-order PE never stalls waiting for this pair's exp/mult
                if p > 0:
                    _emit_pv(p - 1)
            # after the last pair of this group, PV for it is still pending;
            # it is emitted in the next group (or flushed after the loop)

        _emit_pv(NG * NPAIR - 1)


    nc.finalize()
    _CACHED_NC = nc
    return nc


def kernel(q, k, v, pos_proj_w, pos_proj_b, ln1_g, ln1_b, w1, b1,
           ln2_g, ln2_b, w2, b2, ln3_g, ln3_b, w3, b3):
    q = np.asarray(q, dtype=np.float32)
    k = np.asarray(k, dtype=np.float32)
    v = np.asarray(v, dtype=np.float32)

    E = _host_pos_mlp(
        np.asarray(pos_proj_w, np.float64), np.asarray(pos_proj_b, np.float64),
        np.asarray(ln1_g, np.float64), np.asarray(ln1_b, np.float64),
        np.asarray(w1, np.float64), np.asarray(b1, np.float64),
        np.asarray(ln2_g, np.float64), np.asarray(ln2_b, np.float64),
        np.asarray(w2, np.float64), np.asarray(b2, np.float64),
        np.asarray(ln3_g, np.float64), np.asarray(ln3_b, np.float64),
        np.asarray(w3, np.float64), np.asarray(b3, np.float64)).astype(np.float16)

    in_maps = []
    for c in range(NCORES):
        sl = slice(c * W, (c + 1) * W)
        qh = q[sl].reshape(W, NTOK, 4, 2, HD)   # (w, t, i, p, d)
        qp = np.ascontiguousarray(
            qh.transpose(0, 3, 4, 2, 1).reshape(NG, GRP, 128, 196)
            .transpose(0, 2, 1, 3).reshape(NG, 128, GRP * 196).astype(np.float16))
        kh = k[sl].reshape(W, NTOK, 4, 2, HD)
        kpe = np.ascontiguousarray(
            kh[:, :, :, 0, :].transpose(0, 3, 2, 1).reshape(NG, GRP, 64, 196)
            .transpose(0, 2, 1, 3).reshape(NG, 64, GRP * 196).astype(np.float16))
        kpo = np.ascontiguousarray(
            kh[:, :, :, 1, :].transpose(0, 3, 2, 1).reshape(NG, GRP, 64, 196)
            .transpose(0, 2, 1, 3).reshape(NG, 64, GRP * 196).astype(np.float16))
        vh = v[sl].reshape(W, NTOK, 4, 2, HD)   # (w, j, i, p, d)
        ones = np.ones((W, NTOK, 4, 1), np.float32)
        vpe = np.ascontiguousarray(np.concatenate(
            [vh[:, :, :, 0, :], ones], axis=3).reshape(NG, GRP, NTOK, 260)
            .transpose(0, 2, 1, 3).reshape(NG, NTOK, GRP * 260).astype(np.float16))
        vpo = np.ascontiguousarray(np.concatenate(
            [vh[:, :, :, 1, :], ones], axis=3).reshape(NG, GRP, NTOK, 260)
            .transpose(0, 2, 1, 3).reshape(NG, NTOK, GRP * 260).astype(np.float16))
        in_maps.append({"qp": qp, "kpe": kpe, "kpo": kpo, "vpe": vpe, "vpo": vpo,
                        "etab": E})

    nc = _build_nc()
    res = run_bass_kernel_spmd(nc, in_maps, core_ids=list(range(NCORES)))
    global LAST_RESULTS
    LAST_RESULTS = res

    outs = []
    for r in res.results:
        ot = np.asarray(r["ot"]).reshape(NG, NTOK, GRP, 520)
        ot = ot.transpose(0, 2, 1, 3).reshape(W, NTOK, 4, 2, 65).astype(np.float32)
        num = ot[..., 0:64]                    # (w, t, i, p, d)
        den = ot[..., 64:65]
        outs.append((num / den).reshape(W, NTOK, C))
    return np.ascontiguousarray(np.concatenate(outs, axis=0))
